# revision 1
# baseline (speedup 1.0000x reference)
"""Trainium2 Bass kernel for nn_DistanceProbeAlternative (retrieval_knn).

Computes, per batch b:
    proj = emb[b] @ W.T                      # [S, R]
    dist[i, j] = ||proj_i||^2 - 2 proj_i . proj_j + ||proj_j||^2

Sharding: data-parallel over batch B=32 across 8 cores (4 batches/core).
W is replicated. No collectives.

Per-core dataflow:
  1. SWDGE cast-DMA emb in (fp32 HBM -> fp16 SBUF, [128, 2048] chunks).
  2. PE-transpose fp16 128x128 blocks -> fp16 PSUM -> DVE copy to embT [d, s].
  3. projT[r, s] = sum_k WT_k.T @ embT_k (fp16 -> fp32 PSUM); projT kept fp16.
  4. sq = projT^2 (ACT Square from PSUM, f32r); norms: ncol via sq x ones
     (N=2 for fp32r rules), nrow [1,S] * -0.5, replicated to rowrep [128,S]
     (* -2 -> +norms) via K=1 matmul.
  5. dots i-tile = projT_i.T @ projT (fp16, FWL).
  6. Epilogue: ACT Identity tmp = -2*psum + ncol; then outsb = tmp + rowrep
     (3-operand adds split across GPSIMD / DVE); out-DMA on the SP ring.
Batch b's dots interleave with batch b+1's transposes.
"""

import numpy as np
from contextlib import ExitStack

import concourse.bass as bass
import concourse.bacc as bacc
import concourse.tile as tile
from concourse import mybir
from concourse.bass_utils import run_bass_kernel_spmd
from concourse.masks import make_identity

B, S, D, R = 32, 1024, 1024, 128
NCORES = 8
BPC = B // NCORES  # batches per core

F32 = mybir.dt.float32
F32R = mybir.dt.float32r
F16 = mybir.dt.float16
IDENT = mybir.ActivationFunctionType.Identity
SQUARE = mybir.ActivationFunctionType.Square


def build_nc():
    nc = bacc.Bacc("TRN2", target_bir_lowering=False, debug=False)

    emb = nc.dram_tensor("embeddings_batch", [BPC, S, D], F32, kind="ExternalInput")
    Wd = nc.dram_tensor("W", [R, D], F32, kind="ExternalInput")
    out = nc.dram_tensor("out", [BPC, S, S], F32, kind="ExternalOutput")

    NST = S // 128  # 8 s-tiles per batch
    NDT = D // 128  # 8 d-tiles

    with tile.TileContext(nc) as tc, ExitStack() as ctx:
        constp = ctx.enter_context(tc.tile_pool(name="const", bufs=1))
        embin_p = ctx.enter_context(tc.tile_pool(name="embin", bufs=6))
        embT_p = ctx.enter_context(tc.tile_pool(name="embT", bufs=2))
        projT_p = ctx.enter_context(tc.tile_pool(name="projT", bufs=2))
        sq_p = ctx.enter_context(tc.tile_pool(name="sq", bufs=2))
        ncol_p = ctx.enter_context(tc.tile_pool(name="ncol", bufs=2))
        nrow_p = ctx.enter_context(tc.tile_pool(name="nrow", bufs=2))
        rowrep_p = ctx.enter_context(tc.tile_pool(name="rowrep", bufs=2))
        tmp_p = ctx.enter_context(tc.tile_pool(name="tmpsb", bufs=5))
        out_p = ctx.enter_context(tc.tile_pool(name="outsb", bufs=5))
        tpsum_p = ctx.enter_context(tc.tile_pool(name="tpsum", bufs=2, space="PSUM"))
        projps_p = ctx.enter_context(tc.tile_pool(name="projps", bufs=1, space="PSUM"))
        dotps_p = ctx.enter_context(tc.tile_pool(name="dotps", bufs=5, space="PSUM"))

        identityf = constp.tile([128, 128], F32, name="identityf")
        make_identity(nc, identityf)
        identity = constp.tile([128, 128], F16, name="identity")
        nc.vector.tensor_copy(identity, identityf)
        onesf = constp.tile([128, 128], F32, name="onesf")
        nc.gpsimd.memset(onesf, 1.0)
        ones = constp.tile([128, 128], F32R, name="ones")
        nc.vector.tensor_copy(ones, onesf)

        # W: cast to fp16 during DMA, then PE-transpose to WT16
        Wsb = constp.tile([128, D], F16, name="Wsb")
        nc.gpsimd.dma_start(out=Wsb, in_=Wd.ap())
        WT16 = constp.tile([128, D], F16, name="WT16")
        for g in range(NDT // 4):
            wtp = tpsum_p.tile([128, 512], F16, tag="tp", name="wtp")
            for j in range(4):
                k = g * 4 + j
                nc.tensor.transpose(
                    wtp[:, 128 * j : 128 * (j + 1)],
                    Wsb[:, 128 * k : 128 * (k + 1)],
                    identity,
                )
            nc.vector.tensor_copy(WT16[:, 512 * g : 512 * (g + 1)], wtp)

        def quarter_dma(b, q):
            """Cast-DMA in one quarter-batch (2 s-tiles): fp32 HBM -> fp16."""
            esb = embin_p.tile([128, 2048], F16, name="esb")
            src = emb.ap()[b, 256 * q : 256 * (q + 1), :].rearrange(
                "(t p) d -> p t d", p=128
            )
            nc.gpsimd.dma_start(
                out=esb.rearrange("p (t d) -> p t d", t=2), in_=src
            )
            return esb

        def quarter_trans(esb, q, embT):
            """PE-transpose a quarter's 16 fp16 128x128 blocks into embT."""
            embT3 = embT.rearrange("p (k s) -> p k s", k=NDT)
            for t in range(2):
                i = 2 * q + t  # s-tile index
                for g in range(2):  # group of 4 d-chunks
                    tp = tpsum_p.tile([128, 512], F16, tag="tp", name="tp")
                    for j in range(4):
                        k = g * 4 + j
                        nc.tensor.transpose(
                            tp[:, 128 * j : 128 * (j + 1)],
                            esb[:, 1024 * t + 128 * k : 1024 * t + 128 * (k + 1)],
                            identity,
                        )
                    dst = embT3[:, g * 4 : g * 4 + 4, 128 * i : 128 * (i + 1)]
                    tp4 = tp.rearrange("p (k s) -> p k s", k=4)
                    nc.vector.tensor_copy(dst, tp4)

        def proj_phase(embT):
            """16 accumulating matmuls -> projT fp16 + sq f32r."""
            projT = projT_p.tile([128, S], F16, name="projT")
            sq = sq_p.tile([128, S], F32R, name="sq")
            for h in range(2):
                projps = projps_p.tile([128, 512], F32, name="projps")
                for k in range(NDT):
                    nc.tensor.matmul(
                        projps,
                        WT16[:, 128 * k : 128 * (k + 1)],
                        embT[:, S * k + 512 * h : S * k + 512 * (h + 1)],
                        start=(k == 0),
                        stop=(k == NDT - 1),
                    )
                nc.vector.tensor_copy(projT[:, 512 * h : 512 * (h + 1)], projps)
                # sq = projT^2 on ACT, straight from PSUM
                nc.scalar.activation(
                    sq[:, 512 * h : 512 * (h + 1)], projps, SQUARE,
                    bias=0.0, scale=1.0,
                )
            return projT, sq

        def norms_phase(sq):
            """ncol [128, 2/i-tile], nrow [1,S]*-0.5, rowrep [128,S]*+1."""
            # N=2 (ones cols) keeps the fp32r even-count/8B-alignment rules
            ncol_ps = tpsum_p.tile([128, 512], F32, tag="tp", name="ncol_ps")
            for i in range(NST):
                nc.tensor.matmul(
                    ncol_ps[:, 2 * i : 2 * i + 2],
                    sq[:, 128 * i : 128 * (i + 1)],
                    ones[:, 0:2],
                    start=True,
                    stop=True,
                )
            ncol = ncol_p.tile([128, 2 * NST], F32, name="ncol")
            nc.vector.tensor_copy(ncol, ncol_ps[:, 0 : 2 * NST])

            nrow = nrow_p.tile([1, S], F32R, name="nrow")
            for h in range(2):
                nr_ps = tpsum_p.tile([1, 512], F32, tag="tp", name="nr_ps")
                nc.tensor.matmul(
                    nr_ps,
                    ones[:, 0:1],
                    sq[:, 512 * h : 512 * (h + 1)],
                    start=True,
                    stop=True,
                )
                nc.scalar.activation(
                    nrow[0:1, 512 * h : 512 * (h + 1)], nr_ps, IDENT, bias=0.0,
                    scale=-0.5,
                )

            rowrep = rowrep_p.tile([128, S], F32, name="rowrep")
            for h in range(2):
                rp_ps = tpsum_p.tile([128, 512], F32, tag="tp", name="rp_ps")
                nc.tensor.matmul(
                    rp_ps,
                    ones[0:1, 0:128],
                    nrow[0:1, 512 * h : 512 * (h + 1)],
                    start=True,
                    stop=True,
                )
                nc.scalar.activation(
                    rowrep[:, 512 * h : 512 * (h + 1)], rp_ps, IDENT, bias=0.0,
                    scale=-2.0,
                )
            return ncol, rowrep

        def dots_pair(b, pair, projT, ncol, rowrep, last):
            outsb = out_p.tile([128, 2048], F32, name="outsb")
            for t in range(2):
                i = 2 * pair + t
                tmp = tmp_p.tile([128, 1024], F32, name="tmp")
                for h in range(2):
                    d_ps = dotps_p.tile([128, 512], F32, tag="dp", name="d_ps")
                    nc.tensor.matmul(
                        d_ps,
                        projT[:, 128 * i : 128 * (i + 1)],
                        projT[:, 512 * h : 512 * (h + 1)],
                        start=True,
                        stop=True,
                    )
                    nc.scalar.activation(
                        tmp[:, 512 * h : 512 * (h + 1)], d_ps, IDENT,
                        bias=ncol[:, 2 * i : 2 * i + 1], scale=-2.0,
                    )
                # outsb = tmp + norms_row (3-operand; GPSIMD / DVE split)
                # DVE TTs starve under input-DMA bursts (pairs 0-1) and the
                # pair-2 DVE add would queue ahead of the projT copy; so DVE
                # only takes pair-3 adds -- and on the last batch (no
                # prefetch traffic at all) every t=1 add, to shrink the tail.
                if last:
                    add_eng = nc.vector if t == 1 else nc.gpsimd
                else:
                    add_eng = nc.vector if (t == 1 and pair == 3) else nc.gpsimd
                add_eng.tensor_add(
                    outsb[:, 1024 * t : 1024 * (t + 1)], tmp, rowrep
                )
                # per-half out-DMA on the SP ring: the t=0 half starts
                # draining without waiting for the t=1 add
                dram_dst = out.ap()[
                    b, 256 * pair + 128 * t : 256 * pair + 128 * (t + 1), :
                ]
                nc.sync.dma_start(
                    out=dram_dst, in_=outsb[:, 1024 * t : 1024 * (t + 1)]
                )

        # Software pipeline, one batch ahead: batch b+1's input DMAs are all
        # issued at pair 0 of batch b (ahead of the GPSIMD adds in that FIFO),
        # its transposes land during pairs 0-1, proj at pair 2, norms at
        # pair 3 -- so dots(b+1) starts with zero boundary stall.
        embT_next = embT_p.tile([128, NDT * S], F16, name="embT")
        esbs = [quarter_dma(0, q) for q in range(4)]
        for q in range(4):
            quarter_trans(esbs[q], q, embT_next)
        projT, sq = proj_phase(embT_next)
        ncol, rowrep = norms_phase(sq)

        for b in range(BPC):
            last = b + 1 >= BPC
            if not last:
                embT_next = embT_p.tile([128, NDT * S], F16, name="embT")

            for pair in range(NST // 2):
                if not last and pair == 0:
                    esbs = [quarter_dma(b + 1, q) for q in range(4)]
                dots_pair(b, pair, projT, ncol, rowrep, last)
                if not last:
                    if pair < 2:
                        quarter_trans(esbs[2 * pair], 2 * pair, embT_next)
                        quarter_trans(esbs[2 * pair + 1], 2 * pair + 1, embT_next)
                    elif pair == 2:
                        projT_n, sq_n = proj_phase(embT_next)
                    else:
                        ncol_n, rowrep_n = norms_phase(sq_n)

            if not last:
                projT, ncol, rowrep = projT_n, ncol_n, rowrep_n

    nc.finalize()
    return nc


_NC_CACHE = None


def _get_nc():
    global _NC_CACHE
    if _NC_CACHE is None:
        _NC_CACHE = build_nc()
    return _NC_CACHE


def run(embeddings_batch, W, trace=False, tmpdir=None):
    nc = _get_nc()
    emb = np.asarray(embeddings_batch, dtype=np.float32)
    Wf = np.ascontiguousarray(np.asarray(W, dtype=np.float32))
    in_maps = [
        {
            "embeddings_batch": np.ascontiguousarray(emb[c * BPC : (c + 1) * BPC]),
            "W": Wf,
        }
        for c in range(NCORES)
    ]
    res = run_bass_kernel_spmd(
        nc, in_maps, core_ids=list(range(NCORES)), trace=trace, tmpdir=tmpdir
    )
    full = np.concatenate([r["out"] for r in res.results], axis=0)
    return full, res


def kernel(embeddings_batch, W):
    full, _ = run(embeddings_batch, W, trace=False)
    return full

